# revision 26
# baseline (speedup 1.0000x reference)
"""ClusterAwareAttention Trainium2 kernel (8 NeuronCores, axon/PJRT path).

Sharding: data-parallel over (batch, sequence-half) -> 8 shards of 8192 rows.

Two launches, fp8-heavy:
  Pass 1: per-shard cluster pooling partial sums xp = [A|.]^T @ [x|1] in
          fp8e4m3 with DoubleRow matmuls (contraction 256 rows/instr).
          The appended ones-column yields the cluster masses.
  Host:   reduce halves, build pooled K/V constants as fp8 with pow2 scales:
            wk8   = fold(W_q, k_cluster)*SCALE*AL      (logits weights)
            cb8   = cluster_bias*AL (2-head replicated)
            vbd8  = block-diag v_cluster * AV
            wp8   = w_proj * AW
            ibc   = broadcast indicator * 1/(AV*AW)    (bf16)
  Pass 2: per-shard fused attention, transposed (cluster-major) layout,
          n on the free axis in 512-col groups:
            logits^T: fp8 DoubleRow (x part) + fp8 bias matmul -> PSUM
            P = exp(logits*1/AL) -> fp8 [128,2,F] k-tile pairs (ScalarE)
            denom: fp8 DoubleRow indicator matmuls; r = 1/s (DVE, bf16)
            broadcast r via PE; xout: fp8 DoubleRow; normalize on DVE (fp8)
            projection: fp8 DoubleRow -> PSUM -> direct DMA to DRAM f32.
"""

import json
import os
from functools import lru_cache

import numpy as np

import concourse.bass as bass
import concourse.tile as tile
from concourse import mybir
from concourse.bass_utils import run_bass_kernel_spmd

import ml_dtypes

BF16 = ml_dtypes.bfloat16
FP8 = ml_dtypes.float8_e4m3

B, N, C, H, K = 4, 16384, 256, 8, 64
D = C // H
EPS = 1e-8
SCALE = D ** -0.5
NLOC = N // 2           # rows per shard
F = 512                 # group size (n on the free axis)
NGROUPS = NLOC // F
NCORES = 8
CP = 264                # pooling width: 256 x-cols + ones col + pad

f32 = mybir.dt.float32
bf16 = mybir.dt.bfloat16
fp8e4 = mybir.dt.float8e4
DR = mybir.MatmulPerfMode.DoubleRow


# --------------------------------------------------------------------------
# BIR fixup: this container's walrus rejects instructions with >1 sync wait.
# Split extra waits onto single-wait EventSemaphore instructions just before.
# --------------------------------------------------------------------------
def _split_block(bb, counter):
    insts = bb.get("instructions")
    if insts:
        new_insts = []
        for inst in insts:
            si = inst.get("sync_info") or {}
            waits = si.get("on_wait") or []
            if len(waits) > 1:
                for w in waits[:-1]:
                    counter[0] += 1
                    new_insts.append(
                        {
                            "debug": inst.get("debug", 0),
                            "engine": inst["engine"],
                            "ins": [],
                            "name": f"WSPLIT-{counter[0]}",
                            "opcode": "EventSemaphore",
                            "outs": [],
                            "sync_info": {"on_update": [], "on_wait": [w]},
                        }
                    )
                si = dict(si)
                si["on_wait"] = [waits[-1]]
                inst = dict(inst)
                inst["sync_info"] = si
            new_insts.append(inst)
        bb["instructions"] = new_insts
    for sub in bb.get("blocks", []) or []:
        _split_block(sub, counter)


def _fixup_bir_json(bir_json: bytes) -> bytes:
    bir = json.loads(bir_json)
    counter = [0]
    for fn in bir.get("functions", []):
        for bb in fn.get("blocks", []) or []:
            _split_block(bb, counter)
    return json.dumps(bir).encode()


LAST_EXEC_NS = None
TRACE_DIRS = []


def _install_profhook():
    import sys
    import types

    if "antenv.axon_hooks" in sys.modules:
        return
    import antenv

    mod = types.ModuleType("antenv.axon_hooks")
    _hook = [None]
    mod.set_axon_ntff_profile_hook = lambda h: _hook.__setitem__(0, h)
    mod.get_axon_ntff_profile_hook = lambda: _hook[0]
    sys.modules["antenv.axon_hooks"] = mod
    antenv.axon_hooks = mod
    from trn_agent_boot.trn_boot import _ntff_profile_via_ctypes

    mod.set_axon_ntff_profile_hook(
        _ntff_profile_via_ctypes("/opt/axon/libaxon_pjrt.so")
    )


_fixup_installed = False


def _install_fixup():
    global _fixup_installed
    if _fixup_installed:
        return
    _fixup_installed = True
    import concourse.bass_utils as bu
    import concourse.bass2jax as b2j

    orig = bu.compile_bir_kernel

    def patched(bir_json, tmpdir, neff_name="file.neff"):
        return orig(_fixup_bir_json(bir_json), tmpdir, neff_name=neff_name)

    bu.compile_bir_kernel = patched
    b2j.compile_bir_kernel = patched


# --------------------------------------------------------------------------
# Pass 1: xp[kcl, c] = sum_n A[n, kcl] * [x|1][n, c]  (fp8 DoubleRow)
# --------------------------------------------------------------------------
@lru_cache(maxsize=1)
def _build_pass1():
    nc = bass.Bass()
    x_ext = nc.declare_dram_parameter("x", [NLOC, CP], bf16, isOutput=False)
    a_ext = nc.declare_dram_parameter("a", [NLOC, K], bf16, isOutput=False)
    xp_ext = nc.declare_dram_parameter("xp", [K, CP], f32, isOutput=True)

    GT = 8                    # tiles of 128 rows per DMA group
    NG = NLOC // (128 * GT)
    x_r = x_ext.rearrange("(g i p) c -> g p i c", p=128, i=GT)
    a_r = a_ext.rearrange("(g i p) k -> g p i k", p=128, i=GT)

    with tile.TileContext(nc) as tc:
        with (
            tc.tile_pool(name="xin", bufs=3) as xin,
            tc.tile_pool(name="ain", bufs=3) as ain,
            tc.tile_pool(name="acc", bufs=1, space="PSUM") as accp,
            tc.tile_pool(name="outp", bufs=1) as outp,
        ):
            acc = accp.tile([K, CP], f32)
            for g in range(NG):
                xg = xin.tile([128, GT, CP], bf16)
                ag = ain.tile([128, GT, K], bf16)
                nc.sync.dma_start(out=xg[:], in_=x_r[g])
                nc.sync.dma_start(out=ag[:], in_=a_r[g])
                for i in range(GT):
                    t = g * GT + i
                    nc.tensor.matmul(
                        acc[:], ag[:, i, :], xg[:, i, :],
                        start=(t == 0), stop=(t == NLOC // 128 - 1),
                    )
            xps = outp.tile([K, CP], f32)
            nc.vector.tensor_copy(xps[:], acc[:])
            nc.sync.dma_start(out=xp_ext[:], in_=xps[:])
    return nc


# --------------------------------------------------------------------------
# Pass 2: full attention for one shard (fp8 DoubleRow heavy).
# --------------------------------------------------------------------------
@lru_cache(maxsize=4)
def _build_pass2(inv_al: float, zero_bias: bool):
    nc = bass.Bass()
    xs_ext = nc.declare_dram_parameter("xs", [128, 2, NLOC], fp8e4, isOutput=False)
    as_ext = nc.declare_dram_parameter("as_", [K, NLOC], fp8e4, isOutput=False)
    wk_ext = nc.declare_dram_parameter("wk8", [4, 128, 2, 128], fp8e4, isOutput=False)
    cb_ext = nc.declare_dram_parameter("cb8d", [128, 2, 128], fp8e4, isOutput=False)
    ind_ext = nc.declare_dram_parameter("ind8", [2, 128, 2, K], fp8e4, isOutput=False)
    vbd_ext = nc.declare_dram_parameter("vbd8", [2, 128, 2, 128], fp8e4, isOutput=False)
    wp_ext = nc.declare_dram_parameter("wpb", [2, 128, 2, 128], bf16, isOutput=False)
    ibc_ext = nc.declare_dram_parameter("ibc", [K, 2, 128], bf16, isOutput=False)
    bp_ext = nc.declare_dram_parameter("bproj", [2, 128, 1], f32, isOutput=False)
    y_ext = nc.declare_dram_parameter("y", [C, NLOC], bf16, isOutput=True)

    wk_r = wk_ext.rearrange("m p t j -> p m t j")
    ind_r = ind_ext.rearrange("u p t c -> p u t c")
    vbd_r = vbd_ext.rearrange("a p t j -> p a t j")
    wp_r = wp_ext.rearrange("a p t j -> p a t j")
    bp_r = bp_ext.rearrange("a p b -> p a b")

    NQ = 8  # input DMA chunks
    QW = NLOC // NQ

    def _raw_act(eng, out, in_, func, scale=1.0):
        ins = [
            eng.lower_ap(in_),
            mybir.ImmediateValue(dtype=f32, value=0.0),
            mybir.ImmediateValue(dtype=f32, value=scale),
            mybir.ImmediateValue(dtype=f32, value=0.0),
        ]
        return eng.add_instruction(
            mybir.InstActivation(
                name=nc.get_next_instruction_name(),
                func=func,
                ins=ins,
                outs=[eng.lower_ap(out)],
            )
        )

    with tile.TileContext(nc) as tc:
        with (
            tc.tile_pool(name="const", bufs=1) as const,
            tc.tile_pool(name="lgp", bufs=2, space="PSUM") as lgp,
            tc.tile_pool(name="bxp", bufs=2, space="PSUM") as bxp,
            tc.tile_pool(name="ypp", bufs=1, space="PSUM") as ypp,
            tc.tile_pool(name="pp", bufs=4) as pp,
            tc.tile_pool(name="rp", bufs=4) as rp,
            tc.tile_pool(name="bsp", bufs=4) as bsp,
            tc.tile_pool(name="xop", bufs=2) as xop,
            tc.tile_pool(name="ysb", bufs=2) as ysb,
        ):
            xs8 = const.tile([128, 2, NLOC], fp8e4)
            # A^T duplicated 4x (both 64-halves of both k-tiles) so the bias
            # matmul runs in DoubleRow mode; cb8d carries the 1/4 factor.
            as8 = const.tile([128, 2, NLOC], fp8e4)
            wk8 = const.tile([128, 4, 2, 128], fp8e4)
            cb8 = const.tile([128, 2, 128], fp8e4)
            ind8 = const.tile([128, 2, 2, K], fp8e4)
            vbd8 = const.tile([128, 2, 2, 128], fp8e4)
            wp8 = const.tile([128, 2, 2, 128], bf16)
            ibc = const.tile([K, 2, 128], bf16)
            # group-0 critical inputs first, bulk later
            nc.sync.dma_start(out=wk8[:], in_=wk_r[:])
            nc.sync.dma_start(out=cb8[:], in_=cb_ext.rearrange("p t j -> p t j"))
            nc.sync.dma_start(out=xs8[:, :, 0:QW], in_=xs_ext[:, :, 0:QW])
            for t in range(2):
                for hh in range(2):
                    nc.sync.dma_start(
                        out=as8[64 * hh : 64 * (hh + 1), t, 0:QW],
                        in_=as_ext[:, 0:QW],
                    )
            nc.sync.dma_start(out=ind8[:], in_=ind_r[:])
            nc.sync.dma_start(out=vbd8[:], in_=vbd_r[:])
            nc.sync.dma_start(out=wp8[:], in_=wp_r[:])
            nc.sync.dma_start(out=ibc[:], in_=ibc_ext[:])
            if not zero_bias:
                bptc = const.tile([128, 2, 1], f32)
                nc.sync.dma_start(out=bptc[:], in_=bp_r[:])
            for q in range(1, NQ):
                qs_t = slice(q * QW, (q + 1) * QW)
                nc.sync.dma_start(out=xs8[:, :, qs_t], in_=xs_ext[:, :, qs_t])
                for t in range(2):
                    for hh in range(2):
                        nc.sync.dma_start(
                            out=as8[64 * hh : 64 * (hh + 1), t, qs_t],
                            in_=as_ext[:, qs_t],
                        )

            for g in range(NGROUPS):
                n0 = g * F
                xg = xs8[:, :, n0 : n0 + F]
                ag = as8[:, :, n0 : n0 + F]
                # ---- logits^T chunk pairs (+ cluster bias), exp -> fp8 P ----
                P2 = []
                for u in range(2):
                    lg2 = lgp.tile([128, 2, F], f32, tag="lg")
                    for tj in range(2):
                        m = 2 * u + tj
                        nc.tensor.matmul(
                            lg2[:, tj, :], wk8[:, m], xg,
                            start=True, stop=False, perf_mode=DR,
                        )
                        nc.tensor.matmul(
                            lg2[:, tj, :], cb8[:], ag,
                            start=False, stop=True, perf_mode=DR,
                        )
                    p2 = pp.tile([128, 2, F], fp8e4, tag="P")
                    nc.scalar.activation(
                        p2[:], lg2[:],
                        mybir.ActivationFunctionType.Exp,
                        scale=inv_al,
                    )
                    P2.append(p2)
                # ---- softmax denominators ----
                spad = bxp.tile([K, F], f32, tag="bx")
                for u in range(2):
                    nc.tensor.matmul(
                        spad[:], ind8[:, u], P2[u][:],
                        start=(u == 0), stop=(u == 1), perf_mode=DR,
                    )
                # ---- r = 1/s via exp(-ln s) on ScalarE ----
                lns = rp.tile([K, F], f32, tag="lns")
                nc.scalar.activation(
                    lns[:], spad[:], mybir.ActivationFunctionType.Ln
                )
                r = rp.tile([K, F], bf16, tag="r")
                with nc.allow_low_precision(reason="softmax recip bf16"):
                    nc.scalar.activation(
                        r[:], lns[:], mybir.ActivationFunctionType.Exp, scale=-1.0
                    )
                # ---- broadcast r to 32-row blocks (PE), xout, normalize ----
                xo8 = xop.tile([128, 2, F], bf16)
                for t in range(2):
                    Bp = bxp.tile([128, F], f32, tag="bx")
                    nc.tensor.matmul(
                        Bp[:], ibc[:, t, :], r[:], start=True, stop=True,
                    )
                    Bs = bsp.tile([128, F], bf16, tag="Bs")
                    nc.vector.tensor_copy(Bs[:], Bp[:])
                    Xt = bxp.tile([128, F], f32, tag="bx")
                    nc.tensor.matmul(
                        Xt[:], vbd8[:, t], P2[t][:],
                        start=True, stop=True, perf_mode=DR,
                    )
                    nc.vector.tensor_mul(xo8[:, t, :], Xt[:], Bs[:])
                # ---- output projection (transposed out) ----
                yt2 = ypp.tile([128, 2, F], f32, tag="y")
                for cc in range(2):
                    for t in range(2):
                        nc.tensor.matmul(
                            yt2[:, cc, :], wp8[:, cc, t, :], xo8[:, t, :],
                            start=(t == 0), stop=(t == 1),
                        )
                ys2 = ysb.tile([128, 2, F], bf16, tag="ys")
                if zero_bias:
                    nc.vector.tensor_copy(ys2[:], yt2[:])
                else:
                    nc.scalar.activation(
                        ys2[:, 0, :], yt2[:, 0, :],
                        mybir.ActivationFunctionType.Identity,
                        bias=bptc[:, 0, :],
                    )
                    with nc.allow_low_precision(reason="y bias add bf16"):
                        nc.vector.tensor_scalar_add(
                            ys2[:, 1, :], yt2[:, 1, :], bptc[:, 1, :]
                        )
                for cc in range(2):
                    nc.sync.dma_start(
                        out=y_ext[128 * cc : 128 * (cc + 1), n0 : n0 + F],
                        in_=ys2[:, cc, :],
                    )
    return nc


def _pow2(v: float) -> float:
    return float(2.0 ** np.round(np.log2(v)))


# --------------------------------------------------------------------------
# Host orchestration
# --------------------------------------------------------------------------
def kernel(
    voxel_features,
    cluster_assignments,
    w_qkv,
    w_proj,
    b_proj,
    cluster_bias,
):
    _install_fixup()
    x_all = np.ascontiguousarray(np.asarray(voxel_features, dtype=np.float32))
    A_all = np.ascontiguousarray(np.asarray(cluster_assignments, dtype=np.float32))
    w_qkv = np.asarray(w_qkv, dtype=np.float32)
    w_proj_np = np.ascontiguousarray(np.asarray(w_proj, dtype=np.float32))
    b_proj_np = np.asarray(b_proj, dtype=np.float32)
    cb = np.asarray(cluster_bias, dtype=np.float32)

    W_q = w_qkv[:, 0:C]
    W_k = w_qkv[:, C : 2 * C]
    W_v = w_qkv[:, 2 * C : 3 * C]

    trace = bool(os.environ.get("BASS_PROFILE"))
    if trace:
        _install_profhook()
    global LAST_EXEC_NS, TRACE_DIRS
    TRACE_DIRS = []

    # ---------------- pass 1 ----------------
    nc1 = _build_pass1()
    in_maps1 = []
    x8_pool = np.zeros((NCORES, NLOC, CP), BF16)
    a8_pool = np.zeros((NCORES, NLOC, K), BF16)
    for core in range(NCORES):
        b, half = core // 2, core % 2
        sl = slice(half * NLOC, (half + 1) * NLOC)
        x8_pool[core, :, 0:C] = x_all[b, sl].astype(BF16)
        x8_pool[core, :, C] = 1.0
        a8_pool[core] = A_all[b, sl].astype(BF16)
        in_maps1.append({"x": x8_pool[core], "a": a8_pool[core]})
    kw1 = {}
    if trace:
        import tempfile
        d = tempfile.mkdtemp(prefix="p1_trace_")
        TRACE_DIRS.append(d)
        kw1 = dict(trace=True, tmpdir=d)
    res1 = run_bass_kernel_spmd(nc1, in_maps1, list(range(NCORES)), **kw1)
    exec1 = getattr(res1, "exec_time_ns", None)
    xp_parts = np.stack([res1.results[c]["xp"] for c in range(NCORES)])

    # ---------------- host glue ----------------
    # IND2: s-row for chunk j, half h2 is 32*(j//2) + 2*(j%2) + h2; unused
    # rows get a small positive mass so 1/s stays finite.
    eps = 2.0 ** -8
    IND2 = np.full((128, 4, K), eps, np.float32)
    used = np.zeros(K, bool)
    for _j in range(4):
        _c0 = 32 * (_j // 2) + 2 * (_j % 2)
        used[_c0] = used[_c0 + 1] = True
    for _j in range(4):
        _c0 = 32 * (_j // 2) + 2 * (_j % 2)
        IND2[:, _j, used] = 0.0
        IND2[0:64, _j, _c0] = 1.0
        IND2[64:128, _j, _c0 + 1] = 1.0
    IND8 = np.zeros((2, 128, 2, K), FP8)
    for _u in range(2):
        for _t in range(2):
            IND8[_u, :, _t, :] = IND2[:, 2 * _u + _t, :].astype(FP8)

    cb2 = np.zeros((K, 128), np.float32)
    cb2[:, 0:64] = cb
    cb2[:, 64:128] = cb

    Wq3 = W_q.reshape(C, H, D)
    denoms, Wk_cl_all, VBDT_all = [], [], []
    for b in range(B):
        xp = xp_parts[2 * b] + xp_parts[2 * b + 1]
        denom = xp[:, C] + EPS
        denoms.append(denom)
        pooled = xp[:, 0:C] / denom[:, None]
        k_cl = pooled @ W_k
        v_cl = pooled @ W_v
        k3 = k_cl.reshape(K, H, D)
        Wk_cl = np.einsum("chd,khd->chk", Wq3, k3).reshape(C, H * K) * SCALE
        Wk_cl_all.append(Wk_cl)
        v3 = v_cl.reshape(K, H, D)
        VBDT = np.zeros((4, 128, 128), np.float32)
        for j in range(4):
            for h2 in range(2):
                c0 = (j % 2) * 64 + h2 * 32
                VBDT[j, h2 * 64 : (h2 + 1) * 64, c0 : c0 + 32] = v3[
                    :, 2 * j + h2, :
                ]
        VBDT_all.append(VBDT)

    AL = _pow2(
        min(
            4.0 / max(1e-30, max(np.abs(w).max() for w in Wk_cl_all)),
            800.0 / max(1e-30, np.abs(cb).max()),
        )
    )
    AV = _pow2(4.0 / max(1e-30, max(np.abs(v).max() for v in VBDT_all)))
    AW = _pow2(4.0 / max(1e-30, np.abs(w_proj_np).max()))
    cb8d = np.zeros((128, 2, 128), np.float32)
    for _t in range(2):
        cb8d[0:64, _t, :] = cb2 * (AL / 4.0)
        cb8d[64:128, _t, :] = cb2 * (AL / 4.0)
    cb8d = cb8d.astype(FP8)

    # IBC: row 32j+h2 (head h=2j+h2) -> cols 32*(h%4) in tile t=h//4,
    # carrying the 1/(AV*AW) compensation.
    IBC = np.zeros((K, 2, 128), np.float32)
    for _h in range(8):
        _j, _h2 = _h // 2, _h % 2
        _srow = 32 * (_j // 2) + 2 * (_j % 2) + _h2
        IBC[_srow, _h // 4, 32 * (_h % 4) : 32 * (_h % 4) + 32] = 1.0 / (AV * AW)
    IBC = IBC.astype(BF16)

    wk8_all, vbd8_all = [], []
    for b in range(B):
        wk8 = np.zeros((4, 128, 2, 128), FP8)
        Wk_s = (Wk_cl_all[b] * AL).astype(FP8)
        for m in range(4):
            for t in range(2):
                wk8[m, :, t, :] = Wk_s[128 * t : 128 * (t + 1), 128 * m : 128 * (m + 1)]
        wk8_all.append(wk8)
        vbd8 = np.zeros((2, 128, 2, 128), FP8)
        Vs = (np.stack(VBDT_all[b]) * AV).astype(FP8)
        for t in range(2):
            for tp in range(2):
                vbd8[t, :, tp, :] = Vs[2 * t + tp]
        vbd8_all.append(vbd8)

    wpb = np.zeros((2, 128, 2, 128), BF16)
    Wp_s = (w_proj_np * AW).astype(BF16)
    for ccc in range(2):
        for t in range(2):
            wpb[ccc, :, t, :] = Wp_s[128 * t : 128 * (t + 1), 128 * ccc : 128 * (ccc + 1)]

    zero_bias = bool(np.abs(b_proj_np).max() == 0.0)

    # ---------------- pass 2 ----------------
    nc2 = _build_pass2(float(1.0 / AL), zero_bias)
    in_maps2 = []
    for core in range(NCORES):
        b, half = core // 2, core % 2
        sl = slice(half * NLOC, (half + 1) * NLOC)
        xT = x_all[b, sl].T  # (C, NLOC)
        xs8 = np.ascontiguousarray(
            xT.reshape(2, 128, NLOC).transpose(1, 0, 2)
        ).astype(FP8)
        as8 = np.ascontiguousarray(A_all[b, sl].T).astype(FP8)
        in_maps2.append(
            {
                "xs": xs8,
                "as_": as8,
                "wk8": wk8_all[b],
                "cb8d": cb8d,
                "ind8": IND8,
                "vbd8": vbd8_all[b],
                "wpb": wpb,
                "ibc": IBC,
                "bproj": b_proj_np.reshape(2, 128, 1),
            }
        )
    kw2 = {}
    if trace:
        import tempfile
        d = tempfile.mkdtemp(prefix="p2_trace_")
        TRACE_DIRS.append(d)
        kw2 = dict(trace=True, tmpdir=d)
    res2 = run_bass_kernel_spmd(nc2, in_maps2, list(range(NCORES)), **kw2)
    exec2 = getattr(res2, "exec_time_ns", None)
    if exec1 is not None and exec2 is not None:
        LAST_EXEC_NS = exec1 + exec2
        globals()["LAST_EXEC_SPLIT"] = (exec1, exec2)

    y_out = np.zeros((B, N, C), np.float32)
    for core in range(NCORES):
        b, half = core // 2, core % 2
        y_out[b, half * NLOC : (half + 1) * NLOC] = (
            res2.results[core]["y"].astype(np.float32).T
        )
    return y_out


# revision 27
# speedup vs baseline: 1.0707x; 1.0707x over previous
"""ClusterAwareAttention Trainium2 kernel (8 NeuronCores, axon/PJRT path).

Sharding: data-parallel over (batch, sequence-half) -> 8 shards of 8192 rows.

Two launches, fp8-heavy:
  Pass 1: per-shard cluster pooling partial sums xp = [A|.]^T @ [x|1] in
          fp8e4m3 with DoubleRow matmuls (contraction 256 rows/instr).
          The appended ones-column yields the cluster masses.
  Host:   reduce halves, build pooled K/V constants as fp8 with pow2 scales:
            wk8   = fold(W_q, k_cluster)*SCALE*AL      (logits weights)
            cb8   = cluster_bias*AL (2-head replicated)
            vbd8  = block-diag v_cluster * AV
            wp8   = w_proj * AW
            ibc   = broadcast indicator * 1/(AV*AW)    (bf16)
  Pass 2: per-shard fused attention, transposed (cluster-major) layout,
          n on the free axis in 512-col groups:
            logits^T: fp8 DoubleRow (x part) + fp8 bias matmul -> PSUM
            P = exp(logits*1/AL) -> fp8 [128,2,F] k-tile pairs (ScalarE)
            denom: fp8 DoubleRow indicator matmuls; r = 1/s (DVE, bf16)
            broadcast r via PE; xout: fp8 DoubleRow; normalize on DVE (fp8)
            projection: fp8 DoubleRow -> PSUM -> direct DMA to DRAM f32.
"""

import json
import os
from functools import lru_cache

import numpy as np

import concourse.bass as bass
import concourse.tile as tile
from concourse import mybir
from concourse.bass_utils import run_bass_kernel_spmd

import ml_dtypes

BF16 = ml_dtypes.bfloat16
FP8 = ml_dtypes.float8_e4m3

B, N, C, H, K = 4, 16384, 256, 8, 64
D = C // H
EPS = 1e-8
SCALE = D ** -0.5
NLOC = N // 2           # rows per shard
F = 512                 # group size (n on the free axis)
NGROUPS = NLOC // F
NCORES = 8
CP = 264                # pooling width: 256 x-cols + ones col + pad

f32 = mybir.dt.float32
bf16 = mybir.dt.bfloat16
fp8e4 = mybir.dt.float8e4
DR = mybir.MatmulPerfMode.DoubleRow


# --------------------------------------------------------------------------
# BIR fixup: this container's walrus rejects instructions with >1 sync wait.
# Split extra waits onto single-wait EventSemaphore instructions just before.
# --------------------------------------------------------------------------
def _split_block(bb, counter):
    insts = bb.get("instructions")
    if insts:
        new_insts = []
        for inst in insts:
            si = inst.get("sync_info") or {}
            waits = si.get("on_wait") or []
            if len(waits) > 1:
                for w in waits[:-1]:
                    counter[0] += 1
                    new_insts.append(
                        {
                            "debug": inst.get("debug", 0),
                            "engine": inst["engine"],
                            "ins": [],
                            "name": f"WSPLIT-{counter[0]}",
                            "opcode": "EventSemaphore",
                            "outs": [],
                            "sync_info": {"on_update": [], "on_wait": [w]},
                        }
                    )
                si = dict(si)
                si["on_wait"] = [waits[-1]]
                inst = dict(inst)
                inst["sync_info"] = si
            new_insts.append(inst)
        bb["instructions"] = new_insts
    for sub in bb.get("blocks", []) or []:
        _split_block(sub, counter)


def _fixup_bir_json(bir_json: bytes) -> bytes:
    bir = json.loads(bir_json)
    counter = [0]
    for fn in bir.get("functions", []):
        for bb in fn.get("blocks", []) or []:
            _split_block(bb, counter)
    return json.dumps(bir).encode()


LAST_EXEC_NS = None
TRACE_DIRS = []


def _install_profhook():
    import sys
    import types

    if "antenv.axon_hooks" in sys.modules:
        return
    import antenv

    mod = types.ModuleType("antenv.axon_hooks")
    _hook = [None]
    mod.set_axon_ntff_profile_hook = lambda h: _hook.__setitem__(0, h)
    mod.get_axon_ntff_profile_hook = lambda: _hook[0]
    sys.modules["antenv.axon_hooks"] = mod
    antenv.axon_hooks = mod
    from trn_agent_boot.trn_boot import _ntff_profile_via_ctypes

    mod.set_axon_ntff_profile_hook(
        _ntff_profile_via_ctypes("/opt/axon/libaxon_pjrt.so")
    )


_fixup_installed = False


def _install_fixup():
    global _fixup_installed
    if _fixup_installed:
        return
    _fixup_installed = True
    import concourse.bass_utils as bu
    import concourse.bass2jax as b2j

    orig = bu.compile_bir_kernel

    def patched(bir_json, tmpdir, neff_name="file.neff"):
        return orig(_fixup_bir_json(bir_json), tmpdir, neff_name=neff_name)

    bu.compile_bir_kernel = patched
    b2j.compile_bir_kernel = patched


# --------------------------------------------------------------------------
# Pass 1: xp[kcl, c] = sum_n A[n, kcl] * [x|1][n, c]  (fp8 DoubleRow)
# --------------------------------------------------------------------------
@lru_cache(maxsize=1)
def _build_pass1():
    nc = bass.Bass()
    x_ext = nc.declare_dram_parameter("x", [NLOC, CP], bf16, isOutput=False)
    a_ext = nc.declare_dram_parameter("a", [NLOC, K], bf16, isOutput=False)
    xp_ext = nc.declare_dram_parameter("xp", [K, CP], f32, isOutput=True)

    GT = 8                    # tiles of 128 rows per DMA group
    NG = NLOC // (128 * GT)
    x_r = x_ext.rearrange("(g i p) c -> g p i c", p=128, i=GT)
    a_r = a_ext.rearrange("(g i p) k -> g p i k", p=128, i=GT)

    with tile.TileContext(nc) as tc:
        with (
            tc.tile_pool(name="xin", bufs=3) as xin,
            tc.tile_pool(name="ain", bufs=3) as ain,
            tc.tile_pool(name="acc", bufs=1, space="PSUM") as accp,
            tc.tile_pool(name="outp", bufs=1) as outp,
        ):
            acc = accp.tile([K, CP], f32)
            for g in range(NG):
                xg = xin.tile([128, GT, CP], bf16)
                ag = ain.tile([128, GT, K], bf16)
                nc.sync.dma_start(out=xg[:], in_=x_r[g])
                nc.sync.dma_start(out=ag[:], in_=a_r[g])
                for i in range(GT):
                    t = g * GT + i
                    nc.tensor.matmul(
                        acc[:], ag[:, i, :], xg[:, i, :],
                        start=(t == 0), stop=(t == NLOC // 128 - 1),
                    )
            xps = outp.tile([K, CP], f32)
            nc.vector.tensor_copy(xps[:], acc[:])
            nc.sync.dma_start(out=xp_ext[:], in_=xps[:])
    return nc


# --------------------------------------------------------------------------
# Pass 2: full attention for one shard (fp8 DoubleRow heavy).
# --------------------------------------------------------------------------
@lru_cache(maxsize=4)
def _build_pass2(inv_al: float, zero_bias: bool):
    nc = bass.Bass()
    xs_ext = nc.declare_dram_parameter("xs", [128, 2, NLOC], fp8e4, isOutput=False)
    as_ext = nc.declare_dram_parameter("as_", [K, NLOC], fp8e4, isOutput=False)
    wk_ext = nc.declare_dram_parameter("wk8", [4, 128, 2, 128], fp8e4, isOutput=False)
    cb_ext = nc.declare_dram_parameter("cb8d", [128, 2, 128], fp8e4, isOutput=False)
    ind_ext = nc.declare_dram_parameter("ind8", [2, 128, 2, K], fp8e4, isOutput=False)
    vbd_ext = nc.declare_dram_parameter("vbd8", [2, 128, 2, 128], fp8e4, isOutput=False)
    wp_ext = nc.declare_dram_parameter("wpb", [2, 128, 2, 128], bf16, isOutput=False)
    ibc_ext = nc.declare_dram_parameter("ibc", [K, 2, 128], bf16, isOutput=False)
    bp_ext = nc.declare_dram_parameter("bproj", [2, 128, 1], f32, isOutput=False)
    y_ext = nc.declare_dram_parameter("y", [C, NLOC], bf16, isOutput=True)

    wk_r = wk_ext.rearrange("m p t j -> p m t j")
    ind_r = ind_ext.rearrange("u p t c -> p u t c")
    vbd_r = vbd_ext.rearrange("a p t j -> p a t j")
    wp_r = wp_ext.rearrange("a p t j -> p a t j")
    bp_r = bp_ext.rearrange("a p b -> p a b")

    NQ = 8  # input DMA chunks
    QW = NLOC // NQ

    def _raw_act(eng, out, in_, func, scale=1.0):
        ins = [
            eng.lower_ap(in_),
            mybir.ImmediateValue(dtype=f32, value=0.0),
            mybir.ImmediateValue(dtype=f32, value=scale),
            mybir.ImmediateValue(dtype=f32, value=0.0),
        ]
        return eng.add_instruction(
            mybir.InstActivation(
                name=nc.get_next_instruction_name(),
                func=func,
                ins=ins,
                outs=[eng.lower_ap(out)],
            )
        )

    with tile.TileContext(nc) as tc:
        with (
            tc.tile_pool(name="const", bufs=1) as const,
            tc.tile_pool(name="lgp", bufs=2, space="PSUM") as lgp,
            tc.tile_pool(name="bxp", bufs=2, space="PSUM") as bxp,
            tc.tile_pool(name="ypp", bufs=1, space="PSUM") as ypp,
            tc.tile_pool(name="pp", bufs=4) as pp,
            tc.tile_pool(name="rp", bufs=4) as rp,
            tc.tile_pool(name="bsp", bufs=4) as bsp,
            tc.tile_pool(name="xop", bufs=2) as xop,
            tc.tile_pool(name="ysb", bufs=2) as ysb,
        ):
            xs8 = const.tile([128, 2, NLOC], fp8e4)
            # A^T duplicated 4x (both 64-halves of both k-tiles) so the bias
            # matmul runs in DoubleRow mode; cb8d carries the 1/4 factor.
            as8 = const.tile([128, 2, NLOC], fp8e4)
            wk8 = const.tile([128, 4, 2, 128], fp8e4)
            cb8 = const.tile([128, 2, 128], fp8e4)
            ind8 = const.tile([128, 2, 2, K], fp8e4)
            vbd8 = const.tile([128, 2, 2, 128], fp8e4)
            wp8 = const.tile([128, 2, 2, 128], bf16)
            ibc = const.tile([K, 2, 128], bf16)
            # group-0 critical inputs first, bulk later
            nc.sync.dma_start(out=wk8[:], in_=wk_r[:])
            nc.sync.dma_start(out=cb8[:], in_=cb_ext.rearrange("p t j -> p t j"))
            nc.sync.dma_start(out=xs8[:, :, 0:QW], in_=xs_ext[:, :, 0:QW])
            for t in range(2):
                for hh in range(2):
                    nc.sync.dma_start(
                        out=as8[64 * hh : 64 * (hh + 1), t, 0:QW],
                        in_=as_ext[:, 0:QW],
                    )
            nc.sync.dma_start(out=ind8[:], in_=ind_r[:])
            nc.sync.dma_start(out=vbd8[:], in_=vbd_r[:])
            nc.sync.dma_start(out=wp8[:], in_=wp_r[:])
            nc.sync.dma_start(out=ibc[:], in_=ibc_ext[:])
            if not zero_bias:
                bptc = const.tile([128, 2, 1], f32)
                nc.sync.dma_start(out=bptc[:], in_=bp_r[:])
            for q in range(1, NQ):
                qs_t = slice(q * QW, (q + 1) * QW)
                nc.sync.dma_start(out=xs8[:, :, qs_t], in_=xs_ext[:, :, qs_t])
                for t in range(2):
                    for hh in range(2):
                        nc.sync.dma_start(
                            out=as8[64 * hh : 64 * (hh + 1), t, qs_t],
                            in_=as_ext[:, qs_t],
                        )

            for g in range(NGROUPS):
                n0 = g * F
                xg = xs8[:, :, n0 : n0 + F]
                ag = as8[:, :, n0 : n0 + F]
                # ---- logits^T chunk pairs (+ cluster bias), exp -> fp8 P ----
                P2 = []
                for u in range(2):
                    lg2 = lgp.tile([128, 2, F], f32, tag="lg")
                    for tj in range(2):
                        m = 2 * u + tj
                        nc.tensor.matmul(
                            lg2[:, tj, :], wk8[:, m], xg,
                            start=True, stop=False, perf_mode=DR,
                        )
                        nc.tensor.matmul(
                            lg2[:, tj, :], cb8[:], ag,
                            start=False, stop=True, perf_mode=DR,
                        )
                    p2 = pp.tile([128, 2, F], fp8e4, tag="P")
                    nc.scalar.activation(
                        p2[:], lg2[:],
                        mybir.ActivationFunctionType.Exp,
                        scale=inv_al,
                    )
                    P2.append(p2)
                # ---- softmax denominators ----
                spad = bxp.tile([K, F], f32, tag="bx")
                for u in range(2):
                    nc.tensor.matmul(
                        spad[:], ind8[:, u], P2[u][:],
                        start=(u == 0), stop=(u == 1), perf_mode=DR,
                    )
                # ---- r = 1/s via exp(-ln s) on ScalarE ----
                lns = rp.tile([K, F], f32, tag="lns")
                nc.scalar.activation(
                    lns[:], spad[:], mybir.ActivationFunctionType.Ln
                )
                r = rp.tile([K, F], bf16, tag="r")
                with nc.allow_low_precision(reason="softmax recip bf16"):
                    nc.scalar.activation(
                        r[:], lns[:], mybir.ActivationFunctionType.Exp, scale=-1.0
                    )
                # ---- broadcast r to 32-row blocks (PE), xout, normalize ----
                xo8 = xop.tile([128, 2, F], bf16)
                for t in range(2):
                    Bp = bxp.tile([128, F], f32, tag="bx")
                    nc.tensor.matmul(
                        Bp[:], ibc[:, t, :], r[:], start=True, stop=True,
                    )
                    Bs = bsp.tile([128, F], bf16, tag="Bs")
                    if t == 0:
                        nc.scalar.copy(Bs[:], Bp[:])
                    else:
                        nc.vector.tensor_copy(Bs[:], Bp[:])
                    Xt = bxp.tile([128, F], f32, tag="bx")
                    nc.tensor.matmul(
                        Xt[:], vbd8[:, t], P2[t][:],
                        start=True, stop=True, perf_mode=DR,
                    )
                    nc.vector.tensor_mul(xo8[:, t, :], Xt[:], Bs[:])
                # ---- output projection (transposed out) ----
                yt2 = ypp.tile([128, 2, F], f32, tag="y")
                for cc in range(2):
                    for t in range(2):
                        nc.tensor.matmul(
                            yt2[:, cc, :], wp8[:, cc, t, :], xo8[:, t, :],
                            start=(t == 0), stop=(t == 1),
                        )
                ys2 = ysb.tile([128, 2, F], bf16, tag="ys")
                if zero_bias:
                    nc.vector.tensor_copy(ys2[:], yt2[:])
                else:
                    nc.scalar.activation(
                        ys2[:, 0, :], yt2[:, 0, :],
                        mybir.ActivationFunctionType.Identity,
                        bias=bptc[:, 0, :],
                    )
                    with nc.allow_low_precision(reason="y bias add bf16"):
                        nc.vector.tensor_scalar_add(
                            ys2[:, 1, :], yt2[:, 1, :], bptc[:, 1, :]
                        )
                for cc in range(2):
                    nc.sync.dma_start(
                        out=y_ext[128 * cc : 128 * (cc + 1), n0 : n0 + F],
                        in_=ys2[:, cc, :],
                    )
    return nc


def _pow2(v: float) -> float:
    return float(2.0 ** np.round(np.log2(v)))


# --------------------------------------------------------------------------
# Host orchestration
# --------------------------------------------------------------------------
def kernel(
    voxel_features,
    cluster_assignments,
    w_qkv,
    w_proj,
    b_proj,
    cluster_bias,
):
    _install_fixup()
    x_all = np.ascontiguousarray(np.asarray(voxel_features, dtype=np.float32))
    A_all = np.ascontiguousarray(np.asarray(cluster_assignments, dtype=np.float32))
    w_qkv = np.asarray(w_qkv, dtype=np.float32)
    w_proj_np = np.ascontiguousarray(np.asarray(w_proj, dtype=np.float32))
    b_proj_np = np.asarray(b_proj, dtype=np.float32)
    cb = np.asarray(cluster_bias, dtype=np.float32)

    W_q = w_qkv[:, 0:C]
    W_k = w_qkv[:, C : 2 * C]
    W_v = w_qkv[:, 2 * C : 3 * C]

    trace = bool(os.environ.get("BASS_PROFILE"))
    if trace:
        _install_profhook()
    global LAST_EXEC_NS, TRACE_DIRS
    TRACE_DIRS = []

    # ---------------- pass 1 ----------------
    nc1 = _build_pass1()
    in_maps1 = []
    x8_pool = np.zeros((NCORES, NLOC, CP), BF16)
    a8_pool = np.zeros((NCORES, NLOC, K), BF16)
    for core in range(NCORES):
        b, half = core // 2, core % 2
        sl = slice(half * NLOC, (half + 1) * NLOC)
        x8_pool[core, :, 0:C] = x_all[b, sl].astype(BF16)
        x8_pool[core, :, C] = 1.0
        a8_pool[core] = A_all[b, sl].astype(BF16)
        in_maps1.append({"x": x8_pool[core], "a": a8_pool[core]})
    kw1 = {}
    if trace:
        import tempfile
        d = tempfile.mkdtemp(prefix="p1_trace_")
        TRACE_DIRS.append(d)
        kw1 = dict(trace=True, tmpdir=d)
    res1 = run_bass_kernel_spmd(nc1, in_maps1, list(range(NCORES)), **kw1)
    exec1 = getattr(res1, "exec_time_ns", None)
    xp_parts = np.stack([res1.results[c]["xp"] for c in range(NCORES)])

    # ---------------- host glue ----------------
    # IND2: s-row for chunk j, half h2 is 32*(j//2) + 2*(j%2) + h2; unused
    # rows get a small positive mass so 1/s stays finite.
    eps = 2.0 ** -8
    IND2 = np.full((128, 4, K), eps, np.float32)
    used = np.zeros(K, bool)
    for _j in range(4):
        _c0 = 32 * (_j // 2) + 2 * (_j % 2)
        used[_c0] = used[_c0 + 1] = True
    for _j in range(4):
        _c0 = 32 * (_j // 2) + 2 * (_j % 2)
        IND2[:, _j, used] = 0.0
        IND2[0:64, _j, _c0] = 1.0
        IND2[64:128, _j, _c0 + 1] = 1.0
    IND8 = np.zeros((2, 128, 2, K), FP8)
    for _u in range(2):
        for _t in range(2):
            IND8[_u, :, _t, :] = IND2[:, 2 * _u + _t, :].astype(FP8)

    cb2 = np.zeros((K, 128), np.float32)
    cb2[:, 0:64] = cb
    cb2[:, 64:128] = cb

    Wq3 = W_q.reshape(C, H, D)
    denoms, Wk_cl_all, VBDT_all = [], [], []
    for b in range(B):
        xp = xp_parts[2 * b] + xp_parts[2 * b + 1]
        denom = xp[:, C] + EPS
        denoms.append(denom)
        pooled = xp[:, 0:C] / denom[:, None]
        k_cl = pooled @ W_k
        v_cl = pooled @ W_v
        k3 = k_cl.reshape(K, H, D)
        Wk_cl = np.einsum("chd,khd->chk", Wq3, k3).reshape(C, H * K) * SCALE
        Wk_cl_all.append(Wk_cl)
        v3 = v_cl.reshape(K, H, D)
        VBDT = np.zeros((4, 128, 128), np.float32)
        for j in range(4):
            for h2 in range(2):
                c0 = (j % 2) * 64 + h2 * 32
                VBDT[j, h2 * 64 : (h2 + 1) * 64, c0 : c0 + 32] = v3[
                    :, 2 * j + h2, :
                ]
        VBDT_all.append(VBDT)

    AL = _pow2(
        min(
            4.0 / max(1e-30, max(np.abs(w).max() for w in Wk_cl_all)),
            800.0 / max(1e-30, np.abs(cb).max()),
        )
    )
    AV = _pow2(4.0 / max(1e-30, max(np.abs(v).max() for v in VBDT_all)))
    AW = _pow2(4.0 / max(1e-30, np.abs(w_proj_np).max()))
    cb8d = np.zeros((128, 2, 128), np.float32)
    for _t in range(2):
        cb8d[0:64, _t, :] = cb2 * (AL / 4.0)
        cb8d[64:128, _t, :] = cb2 * (AL / 4.0)
    cb8d = cb8d.astype(FP8)

    # IBC: row 32j+h2 (head h=2j+h2) -> cols 32*(h%4) in tile t=h//4,
    # carrying the 1/(AV*AW) compensation.
    IBC = np.zeros((K, 2, 128), np.float32)
    for _h in range(8):
        _j, _h2 = _h // 2, _h % 2
        _srow = 32 * (_j // 2) + 2 * (_j % 2) + _h2
        IBC[_srow, _h // 4, 32 * (_h % 4) : 32 * (_h % 4) + 32] = 1.0 / (AV * AW)
    IBC = IBC.astype(BF16)

    wk8_all, vbd8_all = [], []
    for b in range(B):
        wk8 = np.zeros((4, 128, 2, 128), FP8)
        Wk_s = (Wk_cl_all[b] * AL).astype(FP8)
        for m in range(4):
            for t in range(2):
                wk8[m, :, t, :] = Wk_s[128 * t : 128 * (t + 1), 128 * m : 128 * (m + 1)]
        wk8_all.append(wk8)
        vbd8 = np.zeros((2, 128, 2, 128), FP8)
        Vs = (np.stack(VBDT_all[b]) * AV).astype(FP8)
        for t in range(2):
            for tp in range(2):
                vbd8[t, :, tp, :] = Vs[2 * t + tp]
        vbd8_all.append(vbd8)

    wpb = np.zeros((2, 128, 2, 128), BF16)
    Wp_s = (w_proj_np * AW).astype(BF16)
    for ccc in range(2):
        for t in range(2):
            wpb[ccc, :, t, :] = Wp_s[128 * t : 128 * (t + 1), 128 * ccc : 128 * (ccc + 1)]

    zero_bias = bool(np.abs(b_proj_np).max() == 0.0)

    # ---------------- pass 2 ----------------
    nc2 = _build_pass2(float(1.0 / AL), zero_bias)
    in_maps2 = []
    for core in range(NCORES):
        b, half = core // 2, core % 2
        sl = slice(half * NLOC, (half + 1) * NLOC)
        xT = x_all[b, sl].T  # (C, NLOC)
        xs8 = np.ascontiguousarray(
            xT.reshape(2, 128, NLOC).transpose(1, 0, 2)
        ).astype(FP8)
        as8 = np.ascontiguousarray(A_all[b, sl].T).astype(FP8)
        in_maps2.append(
            {
                "xs": xs8,
                "as_": as8,
                "wk8": wk8_all[b],
                "cb8d": cb8d,
                "ind8": IND8,
                "vbd8": vbd8_all[b],
                "wpb": wpb,
                "ibc": IBC,
                "bproj": b_proj_np.reshape(2, 128, 1),
            }
        )
    kw2 = {}
    if trace:
        import tempfile
        d = tempfile.mkdtemp(prefix="p2_trace_")
        TRACE_DIRS.append(d)
        kw2 = dict(trace=True, tmpdir=d)
    res2 = run_bass_kernel_spmd(nc2, in_maps2, list(range(NCORES)), **kw2)
    exec2 = getattr(res2, "exec_time_ns", None)
    if exec1 is not None and exec2 is not None:
        LAST_EXEC_NS = exec1 + exec2
        globals()["LAST_EXEC_SPLIT"] = (exec1, exec2)

    y_out = np.zeros((B, N, C), np.float32)
    for core in range(NCORES):
        b, half = core // 2, core % 2
        y_out[b, half * NLOC : (half + 1) * NLOC] = (
            res2.results[core]["y"].astype(np.float32).T
        )
    return y_out


# revision 30
# speedup vs baseline: 1.1019x; 1.0292x over previous
"""ClusterAwareAttention Trainium2 kernel (8 NeuronCores, axon/PJRT path).

Sharding: data-parallel over (batch, sequence-half) -> 8 shards of 8192 rows.

Two launches, fp8-heavy:
  Pass 1: per-shard cluster pooling partial sums xp = [A|.]^T @ [x|1] in
          fp8e4m3 with DoubleRow matmuls (contraction 256 rows/instr).
          The appended ones-column yields the cluster masses.
  Host:   reduce halves, build pooled K/V constants as fp8 with pow2 scales:
            wk8   = fold(W_q, k_cluster)*SCALE*AL      (logits weights)
            cb8   = cluster_bias*AL (2-head replicated)
            vbd8  = block-diag v_cluster * AV
            wp8   = w_proj * AW
            ibc   = broadcast indicator * 1/(AV*AW)    (bf16)
  Pass 2: per-shard fused attention, transposed (cluster-major) layout,
          n on the free axis in 512-col groups:
            logits^T: fp8 DoubleRow (x part) + fp8 bias matmul -> PSUM
            P = exp(logits*1/AL) -> fp8 [128,2,F] k-tile pairs (ScalarE)
            denom: fp8 DoubleRow indicator matmuls; r = 1/s (DVE, bf16)
            broadcast r via PE; xout: fp8 DoubleRow; normalize on DVE (fp8)
            projection: fp8 DoubleRow -> PSUM -> direct DMA to DRAM f32.
"""

import json
import os
from functools import lru_cache

import numpy as np

import concourse.bass as bass
import concourse.tile as tile
from concourse import mybir
from concourse.bass_utils import run_bass_kernel_spmd

import ml_dtypes

BF16 = ml_dtypes.bfloat16
FP8 = ml_dtypes.float8_e4m3

B, N, C, H, K = 4, 16384, 256, 8, 64
D = C // H
EPS = 1e-8
SCALE = D ** -0.5
NLOC = N // 2           # rows per shard
F = 512                 # group size (n on the free axis)
NGROUPS = NLOC // F
NCORES = 8
CP = 264                # pooling width: 256 x-cols + ones col + pad

f32 = mybir.dt.float32
bf16 = mybir.dt.bfloat16
fp8e4 = mybir.dt.float8e4
DR = mybir.MatmulPerfMode.DoubleRow


# --------------------------------------------------------------------------
# BIR fixup: this container's walrus rejects instructions with >1 sync wait.
# Split extra waits onto single-wait EventSemaphore instructions just before.
# --------------------------------------------------------------------------
def _split_block(bb, counter):
    insts = bb.get("instructions")
    if insts:
        new_insts = []
        for inst in insts:
            si = inst.get("sync_info") or {}
            waits = si.get("on_wait") or []
            if len(waits) > 1:
                for w in waits[:-1]:
                    counter[0] += 1
                    new_insts.append(
                        {
                            "debug": inst.get("debug", 0),
                            "engine": inst["engine"],
                            "ins": [],
                            "name": f"WSPLIT-{counter[0]}",
                            "opcode": "EventSemaphore",
                            "outs": [],
                            "sync_info": {"on_update": [], "on_wait": [w]},
                        }
                    )
                si = dict(si)
                si["on_wait"] = [waits[-1]]
                inst = dict(inst)
                inst["sync_info"] = si
            new_insts.append(inst)
        bb["instructions"] = new_insts
    for sub in bb.get("blocks", []) or []:
        _split_block(sub, counter)


def _fixup_bir_json(bir_json: bytes) -> bytes:
    bir = json.loads(bir_json)
    counter = [0]
    for fn in bir.get("functions", []):
        for bb in fn.get("blocks", []) or []:
            _split_block(bb, counter)
    return json.dumps(bir).encode()


LAST_EXEC_NS = None
TRACE_DIRS = []


def _install_profhook():
    import sys
    import types

    if "antenv.axon_hooks" in sys.modules:
        return
    import antenv

    mod = types.ModuleType("antenv.axon_hooks")
    _hook = [None]
    mod.set_axon_ntff_profile_hook = lambda h: _hook.__setitem__(0, h)
    mod.get_axon_ntff_profile_hook = lambda: _hook[0]
    sys.modules["antenv.axon_hooks"] = mod
    antenv.axon_hooks = mod
    from trn_agent_boot.trn_boot import _ntff_profile_via_ctypes

    mod.set_axon_ntff_profile_hook(
        _ntff_profile_via_ctypes("/opt/axon/libaxon_pjrt.so")
    )


_fixup_installed = False


def _install_fixup():
    global _fixup_installed
    if _fixup_installed:
        return
    _fixup_installed = True
    import concourse.bass_utils as bu
    import concourse.bass2jax as b2j

    orig = bu.compile_bir_kernel

    def patched(bir_json, tmpdir, neff_name="file.neff"):
        return orig(_fixup_bir_json(bir_json), tmpdir, neff_name=neff_name)

    bu.compile_bir_kernel = patched
    b2j.compile_bir_kernel = patched


# --------------------------------------------------------------------------
# Pass 1: xp[kcl, c] = sum_n A[n, kcl] * [x|1][n, c]  (fp8 DoubleRow)
# --------------------------------------------------------------------------
@lru_cache(maxsize=1)
def _build_pass1():
    nc = bass.Bass()
    x_ext = nc.declare_dram_parameter("x", [NLOC, CP], bf16, isOutput=False)
    a_ext = nc.declare_dram_parameter("a", [NLOC, K], fp8e4, isOutput=False)
    xp_ext = nc.declare_dram_parameter("xp", [K, CP], f32, isOutput=True)

    GT = 8                    # tiles of 128 rows per DMA group
    NG = NLOC // (128 * GT)
    x_r = x_ext.rearrange("(g i p) c -> g p i c", p=128, i=GT)
    a_r = a_ext.rearrange("(g i p) k -> g p i k", p=128, i=GT)

    with tile.TileContext(nc) as tc:
        with (
            tc.tile_pool(name="xin", bufs=3) as xin,
            tc.tile_pool(name="ain", bufs=3) as ain,
            tc.tile_pool(name="acc", bufs=1, space="PSUM") as accp,
            tc.tile_pool(name="outp", bufs=1) as outp,
        ):
            acc = accp.tile([K, CP], f32)
            for g in range(NG):
                xg = xin.tile([128, GT, CP], bf16)
                ag = ain.tile([128, GT, K], fp8e4)
                nc.sync.dma_start(out=xg[:], in_=x_r[g])
                nc.sync.dma_start(out=ag[:], in_=a_r[g])
                for i in range(GT):
                    t = g * GT + i
                    nc.tensor.matmul(
                        acc[:], ag[:, i, :], xg[:, i, :],
                        start=(t == 0), stop=(t == NLOC // 128 - 1),
                    )
            xps = outp.tile([K, CP], f32)
            nc.vector.tensor_copy(xps[:], acc[:])
            nc.sync.dma_start(out=xp_ext[:], in_=xps[:])
    return nc


# --------------------------------------------------------------------------
# Pass 2: full attention for one shard (fp8 DoubleRow heavy).
# --------------------------------------------------------------------------
@lru_cache(maxsize=4)
def _build_pass2(inv_al: float, zero_bias: bool):
    nc = bass.Bass()
    xs_ext = nc.declare_dram_parameter("xs", [128, 2, NLOC], fp8e4, isOutput=False)
    as_ext = nc.declare_dram_parameter("as_", [K, NLOC], fp8e4, isOutput=False)
    wk_ext = nc.declare_dram_parameter("wk8", [4, 128, 2, 128], fp8e4, isOutput=False)
    cb_ext = nc.declare_dram_parameter("cb8d", [128, 2, 128], fp8e4, isOutput=False)
    ind_ext = nc.declare_dram_parameter("ind8", [2, 128, 2, K], fp8e4, isOutput=False)
    vbd_ext = nc.declare_dram_parameter("vbd8", [2, 128, 2, 128], fp8e4, isOutput=False)
    wp_ext = nc.declare_dram_parameter("wpb", [2, 128, 2, 128], bf16, isOutput=False)
    ibc_ext = nc.declare_dram_parameter("ibc", [K, 2, 128], bf16, isOutput=False)
    bp_ext = nc.declare_dram_parameter("bproj", [2, 128, 1], f32, isOutput=False)
    y_ext = nc.declare_dram_parameter("y", [C, NLOC], bf16, isOutput=True)

    wk_r = wk_ext.rearrange("m p t j -> p m t j")
    ind_r = ind_ext.rearrange("u p t c -> p u t c")
    vbd_r = vbd_ext.rearrange("a p t j -> p a t j")
    wp_r = wp_ext.rearrange("a p t j -> p a t j")
    bp_r = bp_ext.rearrange("a p b -> p a b")

    NQ = 8  # input DMA chunks
    QW = NLOC // NQ

    def _raw_act(eng, out, in_, func, scale=1.0):
        ins = [
            eng.lower_ap(in_),
            mybir.ImmediateValue(dtype=f32, value=0.0),
            mybir.ImmediateValue(dtype=f32, value=scale),
            mybir.ImmediateValue(dtype=f32, value=0.0),
        ]
        return eng.add_instruction(
            mybir.InstActivation(
                name=nc.get_next_instruction_name(),
                func=func,
                ins=ins,
                outs=[eng.lower_ap(out)],
            )
        )

    with tile.TileContext(nc) as tc:
        with (
            tc.tile_pool(name="const", bufs=1) as const,
            tc.tile_pool(name="lgp", bufs=2, space="PSUM") as lgp,
            tc.tile_pool(name="bxp", bufs=2, space="PSUM") as bxp,
            tc.tile_pool(name="ypp", bufs=1, space="PSUM") as ypp,
            tc.tile_pool(name="pp", bufs=4) as pp,
            tc.tile_pool(name="rp", bufs=4) as rp,
            tc.tile_pool(name="bsp", bufs=4) as bsp,
            tc.tile_pool(name="xop", bufs=2) as xop,
            tc.tile_pool(name="ysb", bufs=2) as ysb,
        ):
            xs8 = const.tile([128, 2, NLOC], fp8e4)
            # A^T duplicated 4x (both 64-halves of both k-tiles) so the bias
            # matmul runs in DoubleRow mode; cb8d carries the 1/4 factor.
            as8 = const.tile([128, 2, NLOC], fp8e4)
            wk8 = const.tile([128, 4, 2, 128], fp8e4)
            cb8 = const.tile([128, 2, 128], fp8e4)
            ind8 = const.tile([128, 2, 2, K], fp8e4)
            vbd8 = const.tile([128, 2, 2, 128], fp8e4)
            wp8 = const.tile([128, 2, 2, 128], bf16)
            ibc = const.tile([K, 2, 128], bf16)
            # group-0 critical inputs first, bulk later
            nc.sync.dma_start(out=wk8[:], in_=wk_r[:])
            nc.sync.dma_start(out=cb8[:], in_=cb_ext.rearrange("p t j -> p t j"))
            nc.sync.dma_start(out=xs8[:, :, 0:QW], in_=xs_ext[:, :, 0:QW])
            for t in range(2):
                for hh in range(2):
                    nc.sync.dma_start(
                        out=as8[64 * hh : 64 * (hh + 1), t, 0:QW],
                        in_=as_ext[:, 0:QW],
                    )
            nc.sync.dma_start(out=ind8[:], in_=ind_r[:])
            nc.sync.dma_start(out=vbd8[:], in_=vbd_r[:])
            nc.sync.dma_start(out=wp8[:], in_=wp_r[:])
            nc.sync.dma_start(out=ibc[:], in_=ibc_ext[:])
            if not zero_bias:
                bptc = const.tile([128, 2, 1], f32)
                nc.sync.dma_start(out=bptc[:], in_=bp_r[:])
            for q in range(1, NQ):
                qs_t = slice(q * QW, (q + 1) * QW)
                nc.sync.dma_start(out=xs8[:, :, qs_t], in_=xs_ext[:, :, qs_t])
                for t in range(2):
                    for hh in range(2):
                        nc.sync.dma_start(
                            out=as8[64 * hh : 64 * (hh + 1), t, qs_t],
                            in_=as_ext[:, qs_t],
                        )

            for g in range(NGROUPS):
                n0 = g * F
                xg = xs8[:, :, n0 : n0 + F]
                ag = as8[:, :, n0 : n0 + F]
                # ---- logits^T chunk pairs (+ cluster bias), exp -> fp8 P ----
                P2 = []
                for u in range(2):
                    lg2 = lgp.tile([128, 2, F], f32, tag="lg")
                    for tj in range(2):
                        m = 2 * u + tj
                        nc.tensor.matmul(
                            lg2[:, tj, :], wk8[:, m], xg,
                            start=True, stop=False, perf_mode=DR,
                        )
                        nc.tensor.matmul(
                            lg2[:, tj, :], cb8[:], ag,
                            start=False, stop=True, perf_mode=DR,
                        )
                    p2 = pp.tile([128, 2, F], fp8e4, tag="P")
                    nc.scalar.activation(
                        p2[:], lg2[:],
                        mybir.ActivationFunctionType.Exp,
                        scale=inv_al,
                    )
                    P2.append(p2)
                # ---- softmax denominators ----
                spad = bxp.tile([K, F], f32, tag="bx")
                for u in range(2):
                    nc.tensor.matmul(
                        spad[:], ind8[:, u], P2[u][:],
                        start=(u == 0), stop=(u == 1), perf_mode=DR,
                    )
                # ---- r = 1/s via exp(-ln s) on ScalarE ----
                lns = rp.tile([K, F], f32, tag="lns")
                nc.scalar.activation(
                    lns[:], spad[:], mybir.ActivationFunctionType.Ln
                )
                r = rp.tile([K, F], bf16, tag="r")
                with nc.allow_low_precision(reason="softmax recip bf16"):
                    nc.scalar.activation(
                        r[:], lns[:], mybir.ActivationFunctionType.Exp, scale=-1.0
                    )
                # ---- broadcast r to 32-row blocks (PE), xout, normalize ----
                xo8 = xop.tile([128, 2, F], bf16)
                for t in range(2):
                    Bp = bxp.tile([128, F], f32, tag="bx")
                    nc.tensor.matmul(
                        Bp[:], ibc[:, t, :], r[:], start=True, stop=True,
                    )
                    Bs = bsp.tile([128, F], bf16, tag="Bs")
                    if t == 0:
                        nc.scalar.copy(Bs[:], Bp[:])
                    else:
                        nc.vector.tensor_copy(Bs[:], Bp[:])
                    Xt = bxp.tile([128, F], f32, tag="bx")
                    nc.tensor.matmul(
                        Xt[:], vbd8[:, t], P2[t][:],
                        start=True, stop=True, perf_mode=DR,
                    )
                    nc.vector.tensor_mul(xo8[:, t, :], Xt[:], Bs[:])
                # ---- output projection (transposed out) ----
                yt2 = ypp.tile([128, 2, F], f32, tag="y")
                for cc in range(2):
                    for t in range(2):
                        nc.tensor.matmul(
                            yt2[:, cc, :], wp8[:, cc, t, :], xo8[:, t, :],
                            start=(t == 0), stop=(t == 1),
                        )
                ys2 = ysb.tile([128, 2, F], bf16, tag="ys")
                if zero_bias:
                    nc.vector.tensor_copy(ys2[:], yt2[:])
                else:
                    nc.scalar.activation(
                        ys2[:, 0, :], yt2[:, 0, :],
                        mybir.ActivationFunctionType.Identity,
                        bias=bptc[:, 0, :],
                    )
                    with nc.allow_low_precision(reason="y bias add bf16"):
                        nc.vector.tensor_scalar_add(
                            ys2[:, 1, :], yt2[:, 1, :], bptc[:, 1, :]
                        )
                for cc in range(2):
                    nc.sync.dma_start(
                        out=y_ext[128 * cc : 128 * (cc + 1), n0 : n0 + F],
                        in_=ys2[:, cc, :],
                    )
    return nc


def _pow2(v: float) -> float:
    return float(2.0 ** np.round(np.log2(v)))


# --------------------------------------------------------------------------
# Host orchestration
# --------------------------------------------------------------------------
def kernel(
    voxel_features,
    cluster_assignments,
    w_qkv,
    w_proj,
    b_proj,
    cluster_bias,
):
    _install_fixup()
    x_all = np.ascontiguousarray(np.asarray(voxel_features, dtype=np.float32))
    A_all = np.ascontiguousarray(np.asarray(cluster_assignments, dtype=np.float32))
    w_qkv = np.asarray(w_qkv, dtype=np.float32)
    w_proj_np = np.ascontiguousarray(np.asarray(w_proj, dtype=np.float32))
    b_proj_np = np.asarray(b_proj, dtype=np.float32)
    cb = np.asarray(cluster_bias, dtype=np.float32)

    W_q = w_qkv[:, 0:C]
    W_k = w_qkv[:, C : 2 * C]
    W_v = w_qkv[:, 2 * C : 3 * C]

    trace = bool(os.environ.get("BASS_PROFILE"))
    if trace:
        _install_profhook()
    global LAST_EXEC_NS, TRACE_DIRS
    TRACE_DIRS = []

    # ---------------- pass 1 ----------------
    nc1 = _build_pass1()
    in_maps1 = []
    x8_pool = np.zeros((NCORES, NLOC, CP), BF16)
    a8_pool = np.zeros((NCORES, NLOC, K), FP8)
    for core in range(NCORES):
        b, half = core // 2, core % 2
        sl = slice(half * NLOC, (half + 1) * NLOC)
        x8_pool[core, :, 0:C] = x_all[b, sl].astype(BF16)
        x8_pool[core, :, C] = 1.0
        a8_pool[core] = A_all[b, sl].astype(FP8)
        in_maps1.append({"x": x8_pool[core], "a": a8_pool[core]})
    kw1 = {}
    if trace:
        import tempfile
        d = tempfile.mkdtemp(prefix="p1_trace_")
        TRACE_DIRS.append(d)
        kw1 = dict(trace=True, tmpdir=d)
    res1 = run_bass_kernel_spmd(nc1, in_maps1, list(range(NCORES)), **kw1)
    exec1 = getattr(res1, "exec_time_ns", None)
    xp_parts = np.stack([res1.results[c]["xp"] for c in range(NCORES)])

    # ---------------- host glue ----------------
    # IND2: s-row for chunk j, half h2 is 32*(j//2) + 2*(j%2) + h2; unused
    # rows get a small positive mass so 1/s stays finite.
    eps = 2.0 ** -8
    IND2 = np.full((128, 4, K), eps, np.float32)
    used = np.zeros(K, bool)
    for _j in range(4):
        _c0 = 32 * (_j // 2) + 2 * (_j % 2)
        used[_c0] = used[_c0 + 1] = True
    for _j in range(4):
        _c0 = 32 * (_j // 2) + 2 * (_j % 2)
        IND2[:, _j, used] = 0.0
        IND2[0:64, _j, _c0] = 1.0
        IND2[64:128, _j, _c0 + 1] = 1.0
    IND8 = np.zeros((2, 128, 2, K), FP8)
    for _u in range(2):
        for _t in range(2):
            IND8[_u, :, _t, :] = IND2[:, 2 * _u + _t, :].astype(FP8)

    cb2 = np.zeros((K, 128), np.float32)
    cb2[:, 0:64] = cb
    cb2[:, 64:128] = cb

    Wq3 = W_q.reshape(C, H, D)
    denoms, Wk_cl_all, VBDT_all = [], [], []
    for b in range(B):
        xp = xp_parts[2 * b] + xp_parts[2 * b + 1]
        denom = xp[:, C] + EPS
        denoms.append(denom)
        pooled = xp[:, 0:C] / denom[:, None]
        k_cl = pooled @ W_k
        v_cl = pooled @ W_v
        k3 = k_cl.reshape(K, H, D)
        Wk_cl = np.einsum("chd,khd->chk", Wq3, k3).reshape(C, H * K) * SCALE
        Wk_cl_all.append(Wk_cl)
        v3 = v_cl.reshape(K, H, D)
        VBDT = np.zeros((4, 128, 128), np.float32)
        for j in range(4):
            for h2 in range(2):
                c0 = (j % 2) * 64 + h2 * 32
                VBDT[j, h2 * 64 : (h2 + 1) * 64, c0 : c0 + 32] = v3[
                    :, 2 * j + h2, :
                ]
        VBDT_all.append(VBDT)

    AL = _pow2(
        min(
            4.0 / max(1e-30, max(np.abs(w).max() for w in Wk_cl_all)),
            800.0 / max(1e-30, np.abs(cb).max()),
        )
    )
    AV = _pow2(4.0 / max(1e-30, max(np.abs(v).max() for v in VBDT_all)))
    AW = _pow2(4.0 / max(1e-30, np.abs(w_proj_np).max()))
    cb8d = np.zeros((128, 2, 128), np.float32)
    for _t in range(2):
        cb8d[0:64, _t, :] = cb2 * (AL / 4.0)
        cb8d[64:128, _t, :] = cb2 * (AL / 4.0)
    cb8d = cb8d.astype(FP8)

    # IBC: row 32j+h2 (head h=2j+h2) -> cols 32*(h%4) in tile t=h//4,
    # carrying the 1/(AV*AW) compensation.
    IBC = np.zeros((K, 2, 128), np.float32)
    for _h in range(8):
        _j, _h2 = _h // 2, _h % 2
        _srow = 32 * (_j // 2) + 2 * (_j % 2) + _h2
        IBC[_srow, _h // 4, 32 * (_h % 4) : 32 * (_h % 4) + 32] = 1.0 / (AV * AW)
    IBC = IBC.astype(BF16)

    wk8_all, vbd8_all = [], []
    for b in range(B):
        wk8 = np.zeros((4, 128, 2, 128), FP8)
        Wk_s = (Wk_cl_all[b] * AL).astype(FP8)
        for m in range(4):
            for t in range(2):
                wk8[m, :, t, :] = Wk_s[128 * t : 128 * (t + 1), 128 * m : 128 * (m + 1)]
        wk8_all.append(wk8)
        vbd8 = np.zeros((2, 128, 2, 128), FP8)
        Vs = (np.stack(VBDT_all[b]) * AV).astype(FP8)
        for t in range(2):
            for tp in range(2):
                vbd8[t, :, tp, :] = Vs[2 * t + tp]
        vbd8_all.append(vbd8)

    wpb = np.zeros((2, 128, 2, 128), BF16)
    Wp_s = (w_proj_np * AW).astype(BF16)
    for ccc in range(2):
        for t in range(2):
            wpb[ccc, :, t, :] = Wp_s[128 * t : 128 * (t + 1), 128 * ccc : 128 * (ccc + 1)]

    zero_bias = bool(np.abs(b_proj_np).max() == 0.0)

    # ---------------- pass 2 ----------------
    nc2 = _build_pass2(float(1.0 / AL), zero_bias)
    in_maps2 = []
    for core in range(NCORES):
        b, half = core // 2, core % 2
        sl = slice(half * NLOC, (half + 1) * NLOC)
        xT = x_all[b, sl].T  # (C, NLOC)
        xs8 = np.ascontiguousarray(
            xT.reshape(2, 128, NLOC).transpose(1, 0, 2)
        ).astype(FP8)
        as8 = np.ascontiguousarray(A_all[b, sl].T).astype(FP8)
        in_maps2.append(
            {
                "xs": xs8,
                "as_": as8,
                "wk8": wk8_all[b],
                "cb8d": cb8d,
                "ind8": IND8,
                "vbd8": vbd8_all[b],
                "wpb": wpb,
                "ibc": IBC,
                "bproj": b_proj_np.reshape(2, 128, 1),
            }
        )
    kw2 = {}
    if trace:
        import tempfile
        d = tempfile.mkdtemp(prefix="p2_trace_")
        TRACE_DIRS.append(d)
        kw2 = dict(trace=True, tmpdir=d)
    res2 = run_bass_kernel_spmd(nc2, in_maps2, list(range(NCORES)), **kw2)
    exec2 = getattr(res2, "exec_time_ns", None)
    if exec1 is not None and exec2 is not None:
        LAST_EXEC_NS = exec1 + exec2
        globals()["LAST_EXEC_SPLIT"] = (exec1, exec2)

    y_out = np.zeros((B, N, C), np.float32)
    for core in range(NCORES):
        b, half = core // 2, core % 2
        y_out[b, half * NLOC : (half + 1) * NLOC] = (
            res2.results[core]["y"].astype(np.float32).T
        )
    return y_out


# revision 36
# speedup vs baseline: 1.1067x; 1.0043x over previous
"""ClusterAwareAttention Trainium2 kernel (8 NeuronCores, axon/PJRT path).

Sharding: data-parallel over (batch, sequence-half) -> 8 shards of 8192 rows.

Two launches, fp8-heavy:
  Pass 1: per-shard cluster pooling partial sums xp = [A|.]^T @ [x|1] in
          fp8e4m3 with DoubleRow matmuls (contraction 256 rows/instr).
          The appended ones-column yields the cluster masses.
  Host:   reduce halves, build pooled K/V constants as fp8 with pow2 scales:
            wk8   = fold(W_q, k_cluster)*SCALE*AL      (logits weights)
            cb8   = cluster_bias*AL (2-head replicated)
            vbd8  = block-diag v_cluster * AV
            wp8   = w_proj * AW
            ibc   = broadcast indicator * 1/(AV*AW)    (bf16)
  Pass 2: per-shard fused attention, transposed (cluster-major) layout,
          n on the free axis in 512-col groups:
            logits^T: fp8 DoubleRow (x part) + fp8 bias matmul -> PSUM
            P = exp(logits*1/AL) -> fp8 [128,2,F] k-tile pairs (ScalarE)
            denom: fp8 DoubleRow indicator matmuls; r = 1/s (DVE, bf16)
            broadcast r via PE; xout: fp8 DoubleRow; normalize on DVE (fp8)
            projection: fp8 DoubleRow -> PSUM -> direct DMA to DRAM f32.
"""

import json
import os
from functools import lru_cache

import numpy as np

import concourse.bass as bass
import concourse.tile as tile
from concourse import mybir
from concourse.bass_utils import run_bass_kernel_spmd

import ml_dtypes

BF16 = ml_dtypes.bfloat16
FP8 = ml_dtypes.float8_e4m3

B, N, C, H, K = 4, 16384, 256, 8, 64
D = C // H
EPS = 1e-8
SCALE = D ** -0.5
NLOC = N // 2           # rows per shard
F = 512                 # group size (n on the free axis)
NGROUPS = NLOC // F
NCORES = 8
CP = 264                # pooling width: 256 x-cols + ones col + pad

f32 = mybir.dt.float32
bf16 = mybir.dt.bfloat16
fp8e4 = mybir.dt.float8e4
DR = mybir.MatmulPerfMode.DoubleRow


# --------------------------------------------------------------------------
# BIR fixup: this container's walrus rejects instructions with >1 sync wait.
# Split extra waits onto single-wait EventSemaphore instructions just before.
# --------------------------------------------------------------------------
def _split_block(bb, counter):
    insts = bb.get("instructions")
    if insts:
        new_insts = []
        for inst in insts:
            si = inst.get("sync_info") or {}
            waits = si.get("on_wait") or []
            if len(waits) > 1:
                for w in waits[:-1]:
                    counter[0] += 1
                    new_insts.append(
                        {
                            "debug": inst.get("debug", 0),
                            "engine": inst["engine"],
                            "ins": [],
                            "name": f"WSPLIT-{counter[0]}",
                            "opcode": "EventSemaphore",
                            "outs": [],
                            "sync_info": {"on_update": [], "on_wait": [w]},
                        }
                    )
                si = dict(si)
                si["on_wait"] = [waits[-1]]
                inst = dict(inst)
                inst["sync_info"] = si
            new_insts.append(inst)
        bb["instructions"] = new_insts
    for sub in bb.get("blocks", []) or []:
        _split_block(sub, counter)


def _fixup_bir_json(bir_json: bytes) -> bytes:
    bir = json.loads(bir_json)
    counter = [0]
    for fn in bir.get("functions", []):
        for bb in fn.get("blocks", []) or []:
            _split_block(bb, counter)
    return json.dumps(bir).encode()


LAST_EXEC_NS = None
TRACE_DIRS = []


def _install_profhook():
    import sys
    import types

    if "antenv.axon_hooks" in sys.modules:
        return
    import antenv

    mod = types.ModuleType("antenv.axon_hooks")
    _hook = [None]
    mod.set_axon_ntff_profile_hook = lambda h: _hook.__setitem__(0, h)
    mod.get_axon_ntff_profile_hook = lambda: _hook[0]
    sys.modules["antenv.axon_hooks"] = mod
    antenv.axon_hooks = mod
    from trn_agent_boot.trn_boot import _ntff_profile_via_ctypes

    mod.set_axon_ntff_profile_hook(
        _ntff_profile_via_ctypes("/opt/axon/libaxon_pjrt.so")
    )


_fixup_installed = False


def _install_fixup():
    global _fixup_installed
    if _fixup_installed:
        return
    _fixup_installed = True
    import concourse.bass_utils as bu
    import concourse.bass2jax as b2j

    orig = bu.compile_bir_kernel

    def patched(bir_json, tmpdir, neff_name="file.neff"):
        return orig(_fixup_bir_json(bir_json), tmpdir, neff_name=neff_name)

    bu.compile_bir_kernel = patched
    b2j.compile_bir_kernel = patched


# --------------------------------------------------------------------------
# Pass 1: xp[kcl, c] = sum_n A[n, kcl] * [x|1][n, c]  (fp8 DoubleRow)
# --------------------------------------------------------------------------
@lru_cache(maxsize=1)
def _build_pass1():
    nc = bass.Bass()
    x_ext = nc.declare_dram_parameter("x", [NLOC, CP], bf16, isOutput=False)
    a_ext = nc.declare_dram_parameter("a", [NLOC, K], fp8e4, isOutput=False)
    xp_ext = nc.declare_dram_parameter("xp", [K, CP], f32, isOutput=True)

    GT = 8                    # tiles of 128 rows per DMA group
    NG = NLOC // (128 * GT)
    x_r = x_ext.rearrange("(g i p) c -> g p i c", p=128, i=GT)
    a_r = a_ext.rearrange("(g i p) k -> g p i k", p=128, i=GT)

    with tile.TileContext(nc) as tc:
        with (
            tc.tile_pool(name="xin", bufs=3) as xin,
            tc.tile_pool(name="ain", bufs=3) as ain,
            tc.tile_pool(name="acc", bufs=1, space="PSUM") as accp,
            tc.tile_pool(name="outp", bufs=1) as outp,
        ):
            acc = accp.tile([K, CP], f32)
            for g in range(NG):
                xg = xin.tile([128, GT, CP], bf16)
                ag = ain.tile([128, GT, K], fp8e4)
                nc.sync.dma_start(out=xg[:], in_=x_r[g])
                nc.sync.dma_start(out=ag[:], in_=a_r[g])
                for i in range(GT):
                    t = g * GT + i
                    nc.tensor.matmul(
                        acc[:], ag[:, i, :], xg[:, i, :],
                        start=(t == 0), stop=(t == NLOC // 128 - 1),
                    )
            xps = outp.tile([K, CP], f32)
            nc.vector.tensor_copy(xps[:], acc[:])
            nc.sync.dma_start(out=xp_ext[:], in_=xps[:])
    return nc


# --------------------------------------------------------------------------
# Pass 2: full attention for one shard (fp8 DoubleRow heavy).
# --------------------------------------------------------------------------
@lru_cache(maxsize=4)
def _build_pass2(inv_al: float, zero_bias: bool):
    nc = bass.Bass()
    xs_ext = nc.declare_dram_parameter("xs", [128, 2, NLOC], fp8e4, isOutput=False)
    as_ext = nc.declare_dram_parameter("as_", [K, NLOC], fp8e4, isOutput=False)
    wk_ext = nc.declare_dram_parameter("wk8", [4, 128, 2, 128], fp8e4, isOutput=False)
    cb_ext = nc.declare_dram_parameter("cb8d", [128, 2, 128], fp8e4, isOutput=False)
    ind_ext = nc.declare_dram_parameter("ind8", [2, 128, 2, K], fp8e4, isOutput=False)
    vbd_ext = nc.declare_dram_parameter("vbd8", [2, 128, 2, 128], fp8e4, isOutput=False)
    wp_ext = nc.declare_dram_parameter("wpb", [2, 128, 2, 128], bf16, isOutput=False)
    ibc_ext = nc.declare_dram_parameter("ibc", [K, 2, 128], bf16, isOutput=False)
    bp_ext = nc.declare_dram_parameter("bproj", [2, 128, 1], f32, isOutput=False)
    y_ext = nc.declare_dram_parameter("y", [C, NLOC], bf16, isOutput=True)

    wk_r = wk_ext.rearrange("m p t j -> p m t j")
    ind_r = ind_ext.rearrange("u p t c -> p u t c")
    vbd_r = vbd_ext.rearrange("a p t j -> p a t j")
    wp_r = wp_ext.rearrange("a p t j -> p a t j")
    bp_r = bp_ext.rearrange("a p b -> p a b")

    NQ = 8  # input DMA chunks
    QW = NLOC // NQ

    def _raw_act(eng, out, in_, func, scale=1.0):
        ins = [
            eng.lower_ap(in_),
            mybir.ImmediateValue(dtype=f32, value=0.0),
            mybir.ImmediateValue(dtype=f32, value=scale),
            mybir.ImmediateValue(dtype=f32, value=0.0),
        ]
        return eng.add_instruction(
            mybir.InstActivation(
                name=nc.get_next_instruction_name(),
                func=func,
                ins=ins,
                outs=[eng.lower_ap(out)],
            )
        )

    with tile.TileContext(nc) as tc:
        with (
            tc.tile_pool(name="const", bufs=1) as const,
            tc.tile_pool(name="lgp", bufs=2, space="PSUM") as lgp,
            tc.tile_pool(name="bxp", bufs=2, space="PSUM") as bxp,
            tc.tile_pool(name="ypp", bufs=1, space="PSUM") as ypp,
            tc.tile_pool(name="pp", bufs=4) as pp,
            tc.tile_pool(name="rp", bufs=4) as rp,
            tc.tile_pool(name="bsp", bufs=4) as bsp,
            tc.tile_pool(name="xop", bufs=2) as xop,
            tc.tile_pool(name="ysb", bufs=2) as ysb,
        ):
            xs8 = const.tile([128, 2, NLOC], fp8e4)
            # A^T duplicated 4x (both 64-halves of both k-tiles) so the bias
            # matmul runs in DoubleRow mode; cb8d carries the 1/4 factor.
            as8 = const.tile([128, 2, NLOC], fp8e4)
            wk8 = const.tile([128, 4, 2, 128], fp8e4)
            cb8 = const.tile([128, 2, 128], fp8e4)
            ind8 = const.tile([128, 2, 2, K], fp8e4)
            vbd8 = const.tile([128, 2, 2, 128], fp8e4)
            wp8 = const.tile([128, 2, 2, 128], bf16)
            ibc = const.tile([K, 2, 128], bf16)
            # group-0 critical inputs first, bulk later
            nc.sync.dma_start(out=wk8[:], in_=wk_r[:])
            nc.sync.dma_start(out=cb8[:], in_=cb_ext.rearrange("p t j -> p t j"))
            nc.sync.dma_start(out=xs8[:, :, 0:QW], in_=xs_ext[:, :, 0:QW])
            for t in range(2):
                for hh in range(2):
                    nc.sync.dma_start(
                        out=as8[64 * hh : 64 * (hh + 1), t, 0:QW],
                        in_=as_ext[:, 0:QW],
                    )
            nc.sync.dma_start(out=ind8[:], in_=ind_r[:])
            nc.sync.dma_start(out=vbd8[:], in_=vbd_r[:])
            nc.sync.dma_start(out=wp8[:], in_=wp_r[:])
            nc.sync.dma_start(out=ibc[:], in_=ibc_ext[:])
            if not zero_bias:
                bptc = const.tile([128, 2, 1], f32)
                nc.sync.dma_start(out=bptc[:], in_=bp_r[:])
            for q in range(1, NQ):
                qs_t = slice(q * QW, (q + 1) * QW)
                nc.sync.dma_start(out=xs8[:, :, qs_t], in_=xs_ext[:, :, qs_t])
                for t in range(2):
                    for hh in range(2):
                        nc.sync.dma_start(
                            out=as8[64 * hh : 64 * (hh + 1), t, qs_t],
                            in_=as_ext[:, qs_t],
                        )

            for g in range(NGROUPS):
                n0 = g * F
                xg = xs8[:, :, n0 : n0 + F]
                ag = as8[:, :, n0 : n0 + F]
                # ---- logits^T chunk pairs (+ cluster bias), exp -> fp8 P ----
                P2 = []
                for u in range(2):
                    lg2 = lgp.tile([128, 2, F], f32, tag="lg")
                    for tj in range(2):
                        m = 2 * u + tj
                        nc.tensor.matmul(
                            lg2[:, tj, :], wk8[:, m], xg,
                            start=True, stop=False, perf_mode=DR,
                        )
                        nc.tensor.matmul(
                            lg2[:, tj, :], cb8[:], ag,
                            start=False, stop=True, perf_mode=DR,
                        )
                    p2 = pp.tile([128, 2, F], fp8e4, tag="P")
                    nc.scalar.activation(
                        p2[:], lg2[:],
                        mybir.ActivationFunctionType.Exp,
                        scale=inv_al,
                    )
                    P2.append(p2)
                # ---- softmax denominators ----
                spad = bxp.tile([K, F], f32, tag="bx")
                for u in range(2):
                    nc.tensor.matmul(
                        spad[:], ind8[:, u], P2[u][:],
                        start=(u == 0), stop=(u == 1), perf_mode=DR,
                    )
                # ---- r = 1/s via exp(-ln s) on ScalarE ----
                lns = rp.tile([K, F], f32, tag="lns")
                nc.scalar.activation(
                    lns[:], spad[:], mybir.ActivationFunctionType.Ln
                )
                r = rp.tile([K, F], bf16, tag="r")
                with nc.allow_low_precision(reason="softmax recip bf16"):
                    nc.scalar.activation(
                        r[:], lns[:], mybir.ActivationFunctionType.Exp, scale=-1.0
                    )
                # ---- broadcast r to 32-row blocks (PE), xout, normalize ----
                xo8 = xop.tile([128, 2, F], bf16)
                for t in range(2):
                    Bp = bxp.tile([128, F], f32, tag="bx")
                    nc.tensor.matmul(
                        Bp[:], ibc[:, t, :], r[:], start=True, stop=True,
                    )
                    Bs = bsp.tile([128, F], bf16, tag="Bs")
                    if t == 0:
                        nc.scalar.copy(Bs[:], Bp[:])
                    else:
                        nc.vector.tensor_copy(Bs[:], Bp[:])
                    Xt = bxp.tile([128, F], f32, tag="bx")
                    nc.tensor.matmul(
                        Xt[:], vbd8[:, t], P2[t][:],
                        start=True, stop=True, perf_mode=DR,
                    )
                    nc.vector.tensor_mul(xo8[:, t, :], Xt[:], Bs[:])
                # ---- output projection (transposed out) ----
                yt2 = ypp.tile([128, 2, F], f32, tag="y")
                for cc in range(2):
                    for t in range(2):
                        nc.tensor.matmul(
                            yt2[:, cc, :], wp8[:, cc, t, :], xo8[:, t, :],
                            start=(t == 0), stop=(t == 1),
                        )
                ys2 = ysb.tile([128, 2, F], bf16, tag="ys")
                if zero_bias:
                    nc.vector.tensor_copy(ys2[:], yt2[:])
                else:
                    nc.scalar.activation(
                        ys2[:, 0, :], yt2[:, 0, :],
                        mybir.ActivationFunctionType.Identity,
                        bias=bptc[:, 0, :],
                    )
                    with nc.allow_low_precision(reason="y bias add bf16"):
                        nc.vector.tensor_scalar_add(
                            ys2[:, 1, :], yt2[:, 1, :], bptc[:, 1, :]
                        )
                for cc in range(2):
                    nc.sync.dma_start(
                        out=y_ext[128 * cc : 128 * (cc + 1), n0 : n0 + F],
                        in_=ys2[:, cc, :],
                    )
    return nc


def _pow2(v: float) -> float:
    return float(2.0 ** np.round(np.log2(v)))


# --------------------------------------------------------------------------
# Host orchestration
# --------------------------------------------------------------------------
def kernel(
    voxel_features,
    cluster_assignments,
    w_qkv,
    w_proj,
    b_proj,
    cluster_bias,
):
    _install_fixup()
    x_all = np.ascontiguousarray(np.asarray(voxel_features, dtype=np.float32))
    A_all = np.ascontiguousarray(np.asarray(cluster_assignments, dtype=np.float32))
    w_qkv = np.asarray(w_qkv, dtype=np.float32)
    w_proj_np = np.ascontiguousarray(np.asarray(w_proj, dtype=np.float32))
    b_proj_np = np.asarray(b_proj, dtype=np.float32)
    cb = np.asarray(cluster_bias, dtype=np.float32)

    W_q = w_qkv[:, 0:C]
    W_k = w_qkv[:, C : 2 * C]
    W_v = w_qkv[:, 2 * C : 3 * C]

    trace = bool(os.environ.get("BASS_PROFILE"))
    if trace:
        _install_profhook()
    global LAST_EXEC_NS, TRACE_DIRS
    TRACE_DIRS = []

    # ---------------- pass 1 ----------------
    nc1 = _build_pass1()
    in_maps1 = []
    x8_pool = np.zeros((NCORES, NLOC, CP), BF16)
    a8_pool = np.zeros((NCORES, NLOC, K), FP8)
    for core in range(NCORES):
        b, half = core // 2, core % 2
        sl = slice(half * NLOC, (half + 1) * NLOC)
        x8_pool[core, :, 0:C] = x_all[b, sl].astype(BF16)
        x8_pool[core, :, C] = 1.0
        a8_pool[core] = A_all[b, sl].astype(FP8)
        in_maps1.append({"x": x8_pool[core], "a": a8_pool[core]})
    kw1 = {}
    if trace:
        import tempfile
        d = tempfile.mkdtemp(prefix="p1_trace_")
        TRACE_DIRS.append(d)
        kw1 = dict(trace=True, tmpdir=d)
    res1 = run_bass_kernel_spmd(nc1, in_maps1, list(range(NCORES)), **kw1)
    exec1 = getattr(res1, "exec_time_ns", None)
    xp_parts = np.stack([res1.results[c]["xp"] for c in range(NCORES)])

    # ---------------- host glue ----------------
    # IND2: s-row for chunk j, half h2 is 32*(j//2) + 2*(j%2) + h2; unused
    # rows get a small positive mass so 1/s stays finite.
    eps = 2.0 ** -8
    IND2 = np.full((128, 4, K), eps, np.float32)
    used = np.zeros(K, bool)
    for _j in range(4):
        _c0 = 32 * (_j // 2) + 2 * (_j % 2)
        used[_c0] = used[_c0 + 1] = True
    for _j in range(4):
        _c0 = 32 * (_j // 2) + 2 * (_j % 2)
        IND2[:, _j, used] = 0.0
        IND2[0:64, _j, _c0] = 1.0
        IND2[64:128, _j, _c0 + 1] = 1.0
    IND8 = np.zeros((2, 128, 2, K), FP8)
    for _u in range(2):
        for _t in range(2):
            IND8[_u, :, _t, :] = IND2[:, 2 * _u + _t, :].astype(FP8)

    cb2 = np.zeros((K, 128), np.float32)
    cb2[:, 0:64] = cb
    cb2[:, 64:128] = cb

    Wq3 = W_q.reshape(C, H, D)
    denoms, Wk_cl_all, VBDT_all = [], [], []
    for b in range(B):
        xp = xp_parts[2 * b] + xp_parts[2 * b + 1]
        denom = xp[:, C] + EPS
        denoms.append(denom)
        pooled = xp[:, 0:C] / denom[:, None]
        k_cl = pooled @ W_k
        v_cl = pooled @ W_v
        k3 = k_cl.reshape(K, H, D)
        Wk_cl = np.einsum("chd,khd->chk", Wq3, k3).reshape(C, H * K) * SCALE
        Wk_cl_all.append(Wk_cl)
        v3 = v_cl.reshape(K, H, D)
        VBDT = np.zeros((4, 128, 128), np.float32)
        for j in range(4):
            for h2 in range(2):
                c0 = (j % 2) * 64 + h2 * 32
                VBDT[j, h2 * 64 : (h2 + 1) * 64, c0 : c0 + 32] = v3[
                    :, 2 * j + h2, :
                ]
        VBDT_all.append(VBDT)

    AL = _pow2(
        min(
            4.0 / max(1e-30, max(np.abs(w).max() for w in Wk_cl_all)),
            800.0 / max(1e-30, np.abs(cb).max()),
        )
    )
    AV = _pow2(4.0 / max(1e-30, max(np.abs(v).max() for v in VBDT_all)))
    AW = _pow2(4.0 / max(1e-30, np.abs(w_proj_np).max()))
    cb8d = np.zeros((128, 2, 128), np.float32)
    for _t in range(2):
        cb8d[0:64, _t, :] = cb2 * (AL / 4.0)
        cb8d[64:128, _t, :] = cb2 * (AL / 4.0)
    cb8d = cb8d.astype(FP8)

    # IBC: row 32j+h2 (head h=2j+h2) -> cols 32*(h%4) in tile t=h//4,
    # carrying the 1/(AV*AW) compensation.
    IBC = np.zeros((K, 2, 128), np.float32)
    for _h in range(8):
        _j, _h2 = _h // 2, _h % 2
        _srow = 32 * (_j // 2) + 2 * (_j % 2) + _h2
        IBC[_srow, _h // 4, 32 * (_h % 4) : 32 * (_h % 4) + 32] = 1.0 / (AV * AW)
    IBC = IBC.astype(BF16)

    wk8_all, vbd8_all = [], []
    for b in range(B):
        wk8 = np.zeros((4, 128, 2, 128), FP8)
        Wk_s = (Wk_cl_all[b] * AL).astype(FP8)
        for m in range(4):
            for t in range(2):
                wk8[m, :, t, :] = Wk_s[128 * t : 128 * (t + 1), 128 * m : 128 * (m + 1)]
        wk8_all.append(wk8)
        vbd8 = np.zeros((2, 128, 2, 128), FP8)
        Vs = (np.stack(VBDT_all[b]) * AV).astype(FP8)
        for t in range(2):
            for tp in range(2):
                vbd8[t, :, tp, :] = Vs[2 * t + tp]
        vbd8_all.append(vbd8)

    wpb = np.zeros((2, 128, 2, 128), BF16)
    Wp_s = (w_proj_np * AW).astype(BF16)
    for ccc in range(2):
        for t in range(2):
            wpb[ccc, :, t, :] = Wp_s[128 * t : 128 * (t + 1), 128 * ccc : 128 * (ccc + 1)]

    zero_bias = bool(np.abs(b_proj_np).max() == 0.0)

    # ---------------- pass 2 ----------------
    nc2 = _build_pass2(float(1.0 / AL), zero_bias)
    in_maps2 = []
    for core in range(NCORES):
        b, half = core // 2, core % 2
        sl = slice(half * NLOC, (half + 1) * NLOC)
        xT = x_all[b, sl].T  # (C, NLOC)
        xs8 = np.ascontiguousarray(
            xT.reshape(2, 128, NLOC).transpose(1, 0, 2)
        ).astype(FP8)
        as8 = np.ascontiguousarray(A_all[b, sl].T).astype(FP8)
        in_maps2.append(
            {
                "xs": xs8,
                "as_": as8,
                "wk8": wk8_all[b],
                "cb8d": cb8d,
                "ind8": IND8,
                "vbd8": vbd8_all[b],
                "wpb": wpb,
                "ibc": IBC,
                "bproj": b_proj_np.reshape(2, 128, 1),
            }
        )
    kw2 = {}
    if trace:
        import tempfile
        d = tempfile.mkdtemp(prefix="p2_trace_")
        TRACE_DIRS.append(d)
        kw2 = dict(trace=True, tmpdir=d)
    res2 = run_bass_kernel_spmd(nc2, in_maps2, list(range(NCORES)), **kw2)
    exec2 = getattr(res2, "exec_time_ns", None)
    if exec1 is not None and exec2 is not None:
        LAST_EXEC_NS = exec1 + exec2
        globals()["LAST_EXEC_SPLIT"] = (exec1, exec2)

    y_out = np.zeros((B, N, C), np.float32)
    for core in range(NCORES):
        b, half = core // 2, core % 2
        y_out[b, half * NLOC : (half + 1) * NLOC] = (
            res2.results[core]["y"].astype(np.float32).T
        )
    return y_out
